# revision 8
# baseline (speedup 1.0000x reference)
"""Trainium2 Bass kernel for nn_CausalTrajectoryPrediction.

Math (per node n of 64, batch B=1024):
    h1 = relu(x_masked @ W1[n].T)          x_masked = x with col n zeroed
    r1 = relu(h1 @ W2[n].T)
    h3 = relu([r1, x_n] @ W3[n].T + b3[n])
    out[:, n] = relu(h3 @ W4[n] + b4[n])

Restructuring (validated vs the fp32 reference on CPU):
  - The input mask folds into the weights on the host (zero the diagonal
    column of W1[n]); the "own value" path of W3 collapses to one column;
    b3 becomes a ones-row of the layer-3 stationary operand.
  - The last layer is eliminated via w*relu(z) = 0.5*(w*z + w*|z|):
        out[:, n] = 0.5 * relu(c_pos - c_neg)
    where c_pos/c_neg are DVE abs-reduces over two fixed windows of the
    layer-3 PSUM row. Columns are pre-scaled by |W4| and grouped by
    sign(W4) on the host. The linear term a = rep @ (W3ext@W4 + 2*b4*e1)
    is carried by two extra nonnegative columns (v+ in the positive
    window, v- in the negative window; rep is made elementwise >= 0 by
    splitting x_n into x+/x-), so no extra matmul or PSUM tile is needed.
  - All 8 cores share one program, but the sign-split point differs per
    node. Nodes are assigned to program slots sorted by split point; the
    few "middle" columns that are positive on some cores and negative on
    others sit inside the positive window and are duplicated at the tail
    with weight 2 (|z| - 2|z| = -|z|) on cores where they are negative.
  - Sharding: 8 nodes per core (weights are NOT replicated -> 8x less
    HBM traffic), full batch per core. Host gathers (1024, 8) per core.

Per-core dataflow (J = slot 0..7, node = assign[J, core]):
  L1: stationary G1T chunk [64i, 128h], moving xT [64, 512b] -> psum
      [128h, 1024b]; ACT relu -> h1T sbuf bf16.
  L2: stationary W2T chunk [128h, 64m], moving h1T [128, 512b], 4-chunk
      PSUM accumulation -> [64m, 512b]; ACT relu -> rep rows 0:64.
      rep rows 64:67 (x+, x-, ones) are DMA'd from a host array.
  L3: per b-chunk of 128: stationary rep[:, bc*128:+128] [67, 128b],
      moving w3 [67, T] -> psum [128b, T]; two DVE abs-reduces (second
      negated) -> c_pos/c_neg columns.
  The emission interleaves node n's L1 with node n-1's L3 so the PE
  stream stays dense (HAM stays warm).
"""
import sys

sys.path.insert(0, "/opt/trn_rl_repo")

import numpy as np
import ml_dtypes

N_NODES = 64
H = 512
B = 1024
M = 64
N_CORES = 8
JN = 8           # nodes (slots) per core
BC = 8           # batch chunks of 128
BF16 = ml_dtypes.bfloat16

_PROGRAM_CACHE = {}


def _prep(x, W1, W2, W3, b3, W4, b4):
    """Build per-core input maps + program-shape metadata."""
    x = np.asarray(x, np.float32)
    W1 = np.asarray(W1, np.float32)
    W2 = np.asarray(W2, np.float32)
    W3 = np.asarray(W3, np.float32)
    b3 = np.asarray(b3, np.float32)
    W4 = np.asarray(W4, np.float32)
    b4 = np.asarray(b4, np.float32)

    ppos = (W4 >= 0).sum(axis=1)            # sign-split point per node
    order = np.argsort(ppos, kind="stable")
    assign = order.reshape(JN, N_CORES)     # assign[J, core] -> node id
    pmax = np.array([int(ppos[assign[J]].max()) for J in range(JN)])
    pmin = np.array([int(ppos[assign[J]].min()) for J in range(JN)])
    assert pmax.max() < 512, "degenerate all-positive W4 row not supported"
    # symmetric reduce windows: [0:w1) abs+, [w1:2*w1) abs- (zero padded).
    # window1 holds colA + up to pmax scaled columns; window2 holds the
    # remaining 512-P real columns + up to (pmax-P) duplicated middles + colB
    # = 513-P <= 513-pmin columns.
    w1 = np.maximum(1 + pmax, 513 - pmin)
    t = 2 * w1                              # per-slot moving width
    tmax = int(t.max())

    xT = np.ascontiguousarray(x.T).astype(BF16)          # (64, 1024)
    in_maps = []
    for j in range(N_CORES):
        g1 = np.zeros((JN, 64, H), BF16)
        w2 = np.zeros((JN, 128, 4 * M), BF16)
        w3 = np.zeros((JN, 67, tmax), BF16)
        xr = np.zeros((JN, 3, B), BF16)
        for J in range(JN):
            n = int(assign[J, j])
            P = int(ppos[n])
            g1t = W1[n].T.copy()                          # (64 i, 512 h)
            g1t[n, :] = 0.0
            g1[J] = g1t.astype(BF16)
            w2t = W2[n].T                                 # (512 h, 64 m)
            w2[J] = np.ascontiguousarray(
                w2t.reshape(4, 128, M).transpose(1, 0, 2).reshape(128, 4 * M)
            ).astype(BF16)

            w4 = W4[n]
            w3ext = np.zeros((66, H), np.float32)
            w3ext[:64] = W3[n, :, :64].T
            w3ext[64] = W3[n, :, 64 + n]
            w3ext[65] = b3[n]
            scaled = w3ext * np.abs(w4)[None, :]
            pos = np.where(w4 >= 0)[0]
            neg = np.where(w4 < 0)[0]
            nmid = pmax[J] - P
            midc, certain = neg[:nmid], neg[nmid:]

            def lift(c):                                  # (66,k) -> (67,k)
                o = np.zeros((67, c.shape[1]), np.float32)
                o[:64] = c[:64]
                o[64] = c[64]
                o[65] = -c[64]
                o[66] = c[65]
                return o

            v = (w3ext @ w4).astype(np.float32)
            v[65] += 2.0 * b4[n]
            v67 = np.zeros(67, np.float32)
            v67[:64] = v[:64]
            v67[64] = v[64]
            v67[65] = -v[64]
            v67[66] = v[65]

            pad = np.zeros((67, tmax), np.float32)
            pad[:, 0] = np.maximum(v67, 0)                       # colA
            pad[:, 1 : 1 + P] = lift(scaled[:, pos])
            pad[:, 1 + P : 1 + pmax[J]] = lift(scaled[:, midc])
            nc_ = len(certain)
            pad[:, w1[J] : w1[J] + nc_] = lift(scaled[:, certain])
            pad[:, w1[J] + nc_ : w1[J] + nc_ + nmid] = 2.0 * lift(scaled[:, midc])
            pad[:, w1[J] + nc_ + nmid] = np.maximum(-v67, 0)     # colB
            w3[J] = pad.astype(BF16)

            xr[J, 0] = np.maximum(x[:, n], 0).astype(BF16)
            xr[J, 1] = np.maximum(-x[:, n], 0).astype(BF16)
            xr[J, 2] = 1.0
        in_maps.append({"xT": xT, "g1": g1, "w2": w2, "w3": w3, "xr": xr})
    return in_maps, assign, tuple(int(v) for v in w1), tuple(int(v) for v in t), tmax


def _build_program(w1, t, tmax):
    import concourse.bacc as bacc
    import concourse.mybir as mybir
    import concourse.tile as tile

    fp32 = mybir.dt.float32
    bf16 = mybir.dt.bfloat16
    RELU = mybir.ActivationFunctionType.Relu
    ADD = mybir.AluOpType.add
    X = mybir.AxisListType.X

    nc = bacc.Bacc("TRN2", target_bir_lowering=False, debug=False,
                   num_devices=N_CORES)
    xT_d = nc.dram_tensor("xT", [64, B], bf16, kind="ExternalInput")
    g1_d = nc.dram_tensor("g1", [JN, 64, H], bf16, kind="ExternalInput")
    w2_d = nc.dram_tensor("w2", [JN, 128, 4 * M], bf16, kind="ExternalInput")
    w3_d = nc.dram_tensor("w3", [JN, 67, tmax], bf16, kind="ExternalInput")
    xr_d = nc.dram_tensor("xr", [JN, 3, B], bf16, kind="ExternalInput")
    out_d = nc.dram_tensor("out", [B, JN], fp32, kind="ExternalOutput")

    with tile.TileContext(nc) as tc:
        with (
            tc.tile_pool(name="const", bufs=1) as const,
            tc.tile_pool(name="w", bufs=3) as wpool,
            tc.tile_pool(name="act", bufs=2) as apool,
            tc.tile_pool(name="small", bufs=1) as spool,
            tc.tile_pool(name="ps", bufs=4, space="PSUM") as pspool,
        ):
            xT_t = const.tile([64, B], bf16)
            nc.sync.dma_start(xT_t[:], xT_d.ap())

            c2 = spool.tile([128, 128], fp32, tag="c2")

            h1T_t = {}
            rep_t = {}
            w_t = {}

            # HAM warm-up: ~5us of back-to-back matmuls while the first
            # weight DMAs are in flight.
            wu = pspool.tile([64, 512], fp32, tag="ps", name="warmup")
            for _ in range(24):
                nc.tensor.matmul(wu[:], xT_t[:, 0:64], xT_t[:, 0:512],
                                 start=True, stop=True)

            def emit_load(n):
                g1_t = wpool.tile([64, H], bf16, tag="g1", name=f"g1t_{n}")
                nc.sync.dma_start(g1_t[:], g1_d.ap()[n])
                w2_t = wpool.tile([128, 4 * M], bf16, tag="w2", name=f"w2t_{n}")
                nc.sync.dma_start(w2_t[:], w2_d.ap()[n])
                w3_t = wpool.tile([67, tmax], bf16, tag="w3", name=f"w3t_{n}")
                nc.sync.dma_start(w3_t[:], w3_d.ap()[n])
                w_t[n] = (g1_t, w2_t, w3_t)
                rep = apool.tile([67, B], bf16, tag="rep", name=f"rep_{n}")
                nc.sync.dma_start(rep[64:67, :], xr_d.ap()[n])
                rep_t[n] = rep
                h1T_t[n] = apool.tile([128, 4 * B], bf16, tag="h1", name=f"h1T_{n}")

            def emit_l1_unit(n, hc):
                g1_t, _, _ = w_t[n]
                h1T = h1T_t[n]
                ps1 = pspool.tile([128, B], fp32, tag="ps")
                for bc2 in range(2):
                    nc.tensor.matmul(
                        ps1[:, bc2 * 512 : (bc2 + 1) * 512],
                        g1_t[:, hc * 128 : (hc + 1) * 128],
                        xT_t[:, bc2 * 512 : (bc2 + 1) * 512],
                        start=True, stop=True,
                    )
                nc.scalar.activation(h1T[:, hc * B : (hc + 1) * B], ps1[:], RELU)

            def emit_l2_unit(n):
                _, w2_t, _ = w_t[n]
                h1T = h1T_t[n]
                ps2 = pspool.tile([64, B], fp32, tag="ps")
                for hc in range(4):
                    for bc2 in range(2):
                        nc.tensor.matmul(
                            ps2[:, bc2 * 512 : (bc2 + 1) * 512],
                            w2_t[:, hc * M : (hc + 1) * M],
                            h1T[:, hc * B + bc2 * 512 : hc * B + (bc2 + 1) * 512],
                            start=(hc == 0), stop=(hc == 3),
                        )
                nc.scalar.activation(rep_t[n][0:64, :], ps2[:], RELU)

            def emit_l3_unit(n, J, bc):
                _, _, w3_t = w_t[n]
                rep = rep_t[n]
                ps3 = pspool.tile([128, B], fp32, tag="ps")
                stat = rep[:, bc * 128 : (bc + 1) * 128]
                nc.tensor.matmul(ps3[:, 0:512], stat, w3_t[:, 0:512],
                                 start=True, stop=True)
                nc.tensor.matmul(ps3[:, 512 : t[J]], stat, w3_t[:, 512 : t[J]],
                                 start=True, stop=True)
                col = bc * 8 + J
                nc.vector.tensor_reduce(
                    c2[:, 2 * col : 2 * col + 2],
                    ps3[:, 0 : t[J]].rearrange("p (s w) -> p s w", s=2),
                    axis=X, op=ADD, apply_absolute_value=True)

            # software pipeline: interleave node n's L1 with node n-1's L3
            for step in range(JN + 1):
                cur = step if step < JN else None
                prev = step - 1 if step > 0 else None
                if cur is not None:
                    emit_load(cur)
                for i in range(4):
                    if cur is not None:
                        emit_l1_unit(cur, i)
                    if prev is not None:
                        emit_l3_unit(prev, prev, 2 * i)
                        emit_l3_unit(prev, prev, 2 * i + 1)
                if cur is not None:
                    emit_l2_unit(cur)
                if prev is not None:
                    h1T_t.pop(prev, None)
                    w_t.pop(prev, None)
                    rep_t.pop(prev, None)

            # final: out = relu(0.5*(cpos - cneg))
            c3 = c2[:].rearrange("p (c s) -> p c s", s=2)
            t1 = spool.tile([128, 64], fp32, tag="t1")
            nc.vector.tensor_tensor(t1[:], c3[:, :, 0], c3[:, :, 1],
                                    op=mybir.AluOpType.subtract)
            osb = spool.tile([128, 64], fp32, tag="osb")
            nc.scalar.activation(osb[:], t1[:], RELU, scale=0.5)
            nc.sync.dma_start(
                out_d.ap().rearrange("(k p) n -> p k n", p=128),
                osb[:].rearrange("p (k n) -> p k n", k=BC),
            )
    nc.compile()
    return nc


def _get_program(w1, t, tmax):
    key = (w1, t, tmax)
    if key not in _PROGRAM_CACHE:
        _PROGRAM_CACHE[key] = _build_program(w1, t, tmax)
    return _PROGRAM_CACHE[key]


def kernel(x, W1, W2, W3, b3, W4, b4):
    import os
    from concourse.bass_utils import run_bass_kernel_spmd

    in_maps, assign, w1, t, tmax = _prep(x, W1, W2, W3, b3, W4, b4)
    nc = _get_program(w1, t, tmax)

    trace = os.environ.get("CTP_KERNEL_TRACE", "0") == "1"
    kwargs = {}
    if trace:
        import types
        sys.path.insert(0, "/root/.axon_site")
        from trn_agent_boot.trn_boot import _ntff_profile_via_ctypes
        hook = _ntff_profile_via_ctypes("/opt/axon/libaxon_pjrt.so")
        mod = types.ModuleType("antenv.axon_hooks")
        mod.get_axon_ntff_profile_hook = lambda: hook
        mod.set_axon_ntff_profile_hook = lambda h: None
        sys.modules["antenv.axon_hooks"] = mod
        import concourse.bass_utils as bu
        bu.upload_artifacts = lambda tmpdir: f"local:{tmpdir}"
        tdir = os.environ.get("CTP_TRACE_DIR", "/tmp/ctp_trace")
        os.makedirs(tdir, exist_ok=True)
        kwargs = {"trace": True, "tmpdir": tdir}

    res = run_bass_kernel_spmd(nc, in_maps, list(range(N_CORES)), **kwargs)
    if trace:
        print(f"HW exec time: {res.exec_time_ns} ns")

    out = np.zeros((B, N_NODES), np.float32)
    for j in range(N_CORES):
        oj = np.asarray(res.results[j]["out"], np.float32)   # (B, JN)
        for J in range(JN):
            out[:, int(assign[J, j])] = oj[:, J]
    return out
